# revision 4
# baseline (speedup 1.0000x reference)
"""Trainium2 Bass kernel for ModLinear forward:

    alpha = z @ weight_alpha.T + bias_alpha          # [B, IN]
    beta  = z @ weight_beta.T  + bias_beta           # [B, OUT]
    out   = (x * alpha[:, None, :]) @ weight.T + beta[:, None, :]

Key restructuring: alpha modulates the *input channels*, so it folds into the
weight per batch:  out[b] = x[b] @ (weight.T * alpha[b][:, None]) + beta[b].
The huge x tensor is then consumed by a plain matmul with a per-batch
pre-modulated weight (tiny, computed on host along with alpha/beta).

Sharding: x is flattened to [B*N, IN] and split into 8 contiguous row blocks
(one per NeuronCore); batch boundary falls exactly between cores 3 and 4, so
each core uses a single (wmodT, beta) pair. No cross-core communication.

Device kernel per core (rows = 32768):
  for each 128-row block:
    DMA x block [128, 512] -> SBUF
    4x PE-transpose 128x128  -> PSUM   (feature dim onto partitions)
    ACT copy PSUM -> SBUF  (xT chunks)
    4x PE matmul (f32r, accumulate over the 4 feature chunks) -> PSUM [128, 512]
    DVE add beta (broadcast pre-replicated on host) -> SBUF
    DMA out block -> DRAM
"""

import numpy as np

B, N = 2, 131072
IN_F, OUT_F, STYLE_F = 512, 512, 256
NCORES = 8
ROWS = B * N
ROWS_PER_CORE = ROWS // NCORES  # 32768
P = 128


def _build_body(tc, out_ap, x_ap, wt_ap, betar_ap, ident_ap, rows_per_core):
    import concourse.bass as bass
    from concourse import mybir

    nc = tc.nc
    f32 = mybir.dt.float32
    f32r = mybir.dt.float32r
    # Superblock: V*128 rows, packed as [128 partitions, V rows x 512 feat].
    # Partition u holds DRAM rows (V*u .. V*u+V-1) -> V*2 KiB contiguous per
    # partition -> big DMA descriptors, V*512 KiB per dma_start.
    V = 16
    SB = V * P
    nsuper = rows_per_core // SB

    x_v = x_ap.rearrange("(s u v) i -> s u (v i)", u=P, v=V)
    out_v = out_ap.rearrange("(s u v) o -> s u (v o)", u=P, v=V)

    with (
        tc.tile_pool(name="const", bufs=1) as cpool,
        tc.tile_pool(name="xin", bufs=2) as xpool,
        tc.tile_pool(name="xt", bufs=4) as xtpool,
        tc.tile_pool(name="oout", bufs=2) as opool,
        tc.tile_pool(name="ptr", bufs=2, space="PSUM") as ptpool,
        tc.tile_pool(name="pmm", bufs=2, space="PSUM") as pmpool,
    ):
        # Constants: modulated transposed weight (4 chunks of [128, 512] side
        # by side), replicated beta, 128x128 identity for PE transpose.
        wt_sb = cpool.tile([P, 4 * OUT_F], f32r)
        for c in range(4):
            nc.sync.dma_start(
                out=wt_sb[:, c * OUT_F : (c + 1) * OUT_F],
                in_=wt_ap[c * P : (c + 1) * P, :],
            )
        beta_sb = cpool.tile([P, OUT_F], f32)
        nc.sync.dma_start(out=beta_sb[:], in_=betar_ap[:, :])
        ident_sb = cpool.tile([P, P], f32r)
        nc.sync.dma_start(out=ident_sb[:], in_=ident_ap[:, :])

        for s in range(nsuper):
            xt = xpool.tile([P, V * IN_F], f32r)
            nc.sync.dma_start(out=xt[:], in_=x_v[s])
            ot = opool.tile([P, V * OUT_F], f32)

            # 4 passes of 2 row-groups each (PSUM: 2+2 banks, double-buffered)
            for h in range(V // 2):
                pt = ptpool.tile([P, 2 * IN_F], f32r)
                for gg in range(2):
                    v = 2 * h + gg
                    for c in range(4):
                        nc.tensor.transpose(
                            pt[:, gg * IN_F + c * P : gg * IN_F + (c + 1) * P],
                            xt[:, v * IN_F + c * P : v * IN_F + (c + 1) * P],
                            ident_sb[:],
                        )
                xts = xtpool.tile([P, 2 * IN_F], f32r)
                nc.scalar.copy(out=xts[:, :IN_F], in_=pt[:, :IN_F])
                nc.scalar.copy(out=xts[:, IN_F:], in_=pt[:, IN_F:])

                po = pmpool.tile([P, 2 * OUT_F], f32)
                for gg in range(2):
                    for c in range(4):
                        nc.tensor.matmul(
                            po[:, gg * OUT_F : (gg + 1) * OUT_F],
                            xts[:, gg * IN_F + c * P : gg * IN_F + (c + 1) * P],
                            wt_sb[:, c * OUT_F : (c + 1) * OUT_F],
                            start=(c == 0),
                            stop=(c == 3),
                        )

                for gg in range(2):
                    v = 2 * h + gg
                    nc.vector.tensor_add(
                        out=ot[:, v * OUT_F : (v + 1) * OUT_F],
                        in0=po[:, gg * OUT_F : (gg + 1) * OUT_F],
                        in1=beta_sb[:],
                    )

            nc.scalar.dma_start(out=out_v[s], in_=ot[:])


def build_nc(rows_per_core=ROWS_PER_CORE):
    """Build + compile the per-core Bass program. Returns (nc, names)."""
    import concourse.tile as tile
    from concourse import bacc, mybir

    f32 = mybir.dt.float32
    f32r = mybir.dt.float32r
    nc = bacc.Bacc(
        "TRN2", target_bir_lowering=False, debug=False, num_devices=NCORES
    )
    x_t = nc.dram_tensor("x", [rows_per_core, IN_F], f32r, kind="ExternalInput")
    wt_t = nc.dram_tensor("wt", [IN_F, OUT_F], f32r, kind="ExternalInput")
    betar_t = nc.dram_tensor("betar", [P, OUT_F], f32, kind="ExternalInput")
    ident_t = nc.dram_tensor("ident", [P, P], f32r, kind="ExternalInput")
    out_t = nc.dram_tensor("out", [rows_per_core, OUT_F], f32, kind="ExternalOutput")

    with tile.TileContext(nc) as tc:
        _build_body(
            tc, out_t.ap(), x_t.ap(), wt_t.ap(), betar_t.ap(), ident_t.ap(),
            rows_per_core,
        )
    nc.compile()
    return nc


_NC_CACHE = {}


def _get_nc(rows_per_core=ROWS_PER_CORE):
    if rows_per_core not in _NC_CACHE:
        _NC_CACHE[rows_per_core] = build_nc(rows_per_core)
    return _NC_CACHE[rows_per_core]


def host_prep(x, z, weight, weight_alpha, bias_alpha, weight_beta, bias_beta):
    """Compute per-batch modulated weights + biases, and per-core in_maps."""
    z64 = z.astype(np.float64)
    alpha = (z64 @ weight_alpha.astype(np.float64).T) + bias_alpha.astype(np.float64)
    beta = (z64 @ weight_beta.astype(np.float64).T) + bias_beta.astype(np.float64)
    alpha = alpha.astype(np.float32)  # [B, IN_F]
    beta = beta.astype(np.float32)  # [B, OUT_F]

    # wmodT[b][i, o] = weight[o, i] * alpha[b, i]
    wmodT = [
        np.ascontiguousarray(weight.T * alpha[b][:, None]).astype(np.float32)
        for b in range(B)
    ]
    betar = [
        np.ascontiguousarray(np.broadcast_to(beta[b], (P, OUT_F))).astype(np.float32)
        for b in range(B)
    ]
    ident = np.eye(P, dtype=np.float32)

    xf = np.ascontiguousarray(x).reshape(ROWS, IN_F)
    in_maps = []
    for k in range(NCORES):
        b = (k * ROWS_PER_CORE) // N  # batch this core's rows belong to
        in_maps.append(
            {
                "x": xf[k * ROWS_PER_CORE : (k + 1) * ROWS_PER_CORE],
                "wt": wmodT[b],
                "betar": betar[b],
                "ident": ident,
            }
        )
    return in_maps


def kernel(x, z, weight, weight_alpha, bias_alpha, weight_beta, bias_beta,
           _trace=False):
    from concourse.bass_utils import run_bass_kernel_spmd

    in_maps = host_prep(
        x, z, weight, weight_alpha, bias_alpha, weight_beta, bias_beta
    )
    nc = _get_nc()
    res = run_bass_kernel_spmd(
        nc, in_maps, core_ids=list(range(NCORES)), trace=_trace
    )
    out = np.concatenate([res.results[k]["out"] for k in range(NCORES)], axis=0)
    out = out.reshape(B, N, OUT_F)
    if _trace:
        kernel.last_results = res
    return out


# revision 5
# speedup vs baseline: 1.0167x; 1.0167x over previous
"""Trainium2 Bass kernel for ModLinear forward:

    alpha = z @ weight_alpha.T + bias_alpha          # [B, IN]
    beta  = z @ weight_beta.T  + bias_beta           # [B, OUT]
    out   = (x * alpha[:, None, :]) @ weight.T + beta[:, None, :]

Key restructuring: alpha modulates the *input channels*, so it folds into the
weight per batch:  out[b] = x[b] @ (weight.T * alpha[b][:, None]) + beta[b].
The huge x tensor is then consumed by a plain matmul with a per-batch
pre-modulated weight (tiny, computed on host along with alpha/beta).

Sharding: x is flattened to [B*N, IN] and split into 8 contiguous row blocks
(one per NeuronCore); batch boundary falls exactly between cores 3 and 4, so
each core uses a single (wmodT, beta) pair. No cross-core communication.

Device kernel per core (rows = 32768):
  for each 128-row block:
    DMA x block [128, 512] -> SBUF
    4x PE-transpose 128x128  -> PSUM   (feature dim onto partitions)
    ACT copy PSUM -> SBUF  (xT chunks)
    4x PE matmul (f32r, accumulate over the 4 feature chunks) -> PSUM [128, 512]
    DVE add beta (broadcast pre-replicated on host) -> SBUF
    DMA out block -> DRAM
"""

import numpy as np

B, N = 2, 131072
IN_F, OUT_F, STYLE_F = 512, 512, 256
NCORES = 8
ROWS = B * N
ROWS_PER_CORE = ROWS // NCORES  # 32768
P = 128


def _build_body(tc, out_ap, x_ap, wt_ap, betar_ap, ident_ap, rows_per_core):
    import concourse.bass as bass
    from concourse import mybir

    nc = tc.nc
    f32 = mybir.dt.float32
    f32r = mybir.dt.float32r
    # Superblock: V*128 rows, packed as [128 partitions, V rows x 512 feat].
    # Partition u holds DRAM rows (V*u .. V*u+V-1) -> V*2 KiB contiguous per
    # partition -> big DMA descriptors, V*512 KiB per dma_start.
    V = 8
    SB = V * P
    nsuper = rows_per_core // SB

    x_v = x_ap.rearrange("(s u v) i -> s u (v i)", u=P, v=V)
    out_v = out_ap.rearrange("(s u v) o -> s u (v o)", u=P, v=V)

    with (
        tc.tile_pool(name="const", bufs=1) as cpool,
        tc.tile_pool(name="xin", bufs=3) as xpool,
        tc.tile_pool(name="xt", bufs=6) as xtpool,
        tc.tile_pool(name="oout", bufs=3) as opool,
        tc.tile_pool(name="ptr", bufs=2, space="PSUM") as ptpool,
        tc.tile_pool(name="pmm", bufs=2, space="PSUM") as pmpool,
    ):
        # Constants: modulated transposed weight (4 chunks of [128, 512] side
        # by side), replicated beta, 128x128 identity for PE transpose.
        wt_sb = cpool.tile([P, 4 * OUT_F], f32r)
        for c in range(4):
            nc.sync.dma_start(
                out=wt_sb[:, c * OUT_F : (c + 1) * OUT_F],
                in_=wt_ap[c * P : (c + 1) * P, :],
            )
        beta_sb = cpool.tile([P, OUT_F], f32)
        nc.sync.dma_start(out=beta_sb[:], in_=betar_ap[:, :])
        ident_sb = cpool.tile([P, P], f32r)
        nc.sync.dma_start(out=ident_sb[:], in_=ident_ap[:, :])

        for s in range(nsuper):
            xt = xpool.tile([P, V * IN_F], f32r)
            nc.sync.dma_start(out=xt[:], in_=x_v[s])
            ot = opool.tile([P, V * OUT_F], f32)

            # 4 passes of 2 row-groups each (PSUM: 2+2 banks, double-buffered)
            for h in range(V // 2):
                pt = ptpool.tile([P, 2 * IN_F], f32r)
                for gg in range(2):
                    v = 2 * h + gg
                    for c in range(4):
                        nc.tensor.transpose(
                            pt[:, gg * IN_F + c * P : gg * IN_F + (c + 1) * P],
                            xt[:, v * IN_F + c * P : v * IN_F + (c + 1) * P],
                            ident_sb[:],
                        )
                xts = xtpool.tile([P, 2 * IN_F], f32r)
                nc.scalar.copy(out=xts[:, :IN_F], in_=pt[:, :IN_F])
                nc.scalar.copy(out=xts[:, IN_F:], in_=pt[:, IN_F:])

                po = pmpool.tile([P, 2 * OUT_F], f32)
                for gg in range(2):
                    for c in range(4):
                        nc.tensor.matmul(
                            po[:, gg * OUT_F : (gg + 1) * OUT_F],
                            xts[:, gg * IN_F + c * P : gg * IN_F + (c + 1) * P],
                            wt_sb[:, c * OUT_F : (c + 1) * OUT_F],
                            start=(c == 0),
                            stop=(c == 3),
                        )

                for gg in range(2):
                    v = 2 * h + gg
                    nc.vector.tensor_add(
                        out=ot[:, v * OUT_F : (v + 1) * OUT_F],
                        in0=po[:, gg * OUT_F : (gg + 1) * OUT_F],
                        in1=beta_sb[:],
                    )

            nc.scalar.dma_start(out=out_v[s], in_=ot[:])


def build_nc(rows_per_core=ROWS_PER_CORE):
    """Build + compile the per-core Bass program. Returns (nc, names)."""
    import concourse.tile as tile
    from concourse import bacc, mybir

    f32 = mybir.dt.float32
    f32r = mybir.dt.float32r
    nc = bacc.Bacc(
        "TRN2", target_bir_lowering=False, debug=False, num_devices=NCORES
    )
    x_t = nc.dram_tensor("x", [rows_per_core, IN_F], f32r, kind="ExternalInput")
    wt_t = nc.dram_tensor("wt", [IN_F, OUT_F], f32r, kind="ExternalInput")
    betar_t = nc.dram_tensor("betar", [P, OUT_F], f32, kind="ExternalInput")
    ident_t = nc.dram_tensor("ident", [P, P], f32r, kind="ExternalInput")
    out_t = nc.dram_tensor("out", [rows_per_core, OUT_F], f32, kind="ExternalOutput")

    with tile.TileContext(nc) as tc:
        _build_body(
            tc, out_t.ap(), x_t.ap(), wt_t.ap(), betar_t.ap(), ident_t.ap(),
            rows_per_core,
        )
    nc.compile()
    return nc


_NC_CACHE = {}


def _get_nc(rows_per_core=ROWS_PER_CORE):
    if rows_per_core not in _NC_CACHE:
        _NC_CACHE[rows_per_core] = build_nc(rows_per_core)
    return _NC_CACHE[rows_per_core]


def host_prep(x, z, weight, weight_alpha, bias_alpha, weight_beta, bias_beta):
    """Compute per-batch modulated weights + biases, and per-core in_maps."""
    z64 = z.astype(np.float64)
    alpha = (z64 @ weight_alpha.astype(np.float64).T) + bias_alpha.astype(np.float64)
    beta = (z64 @ weight_beta.astype(np.float64).T) + bias_beta.astype(np.float64)
    alpha = alpha.astype(np.float32)  # [B, IN_F]
    beta = beta.astype(np.float32)  # [B, OUT_F]

    # wmodT[b][i, o] = weight[o, i] * alpha[b, i]
    wmodT = [
        np.ascontiguousarray(weight.T * alpha[b][:, None]).astype(np.float32)
        for b in range(B)
    ]
    betar = [
        np.ascontiguousarray(np.broadcast_to(beta[b], (P, OUT_F))).astype(np.float32)
        for b in range(B)
    ]
    ident = np.eye(P, dtype=np.float32)

    xf = np.ascontiguousarray(x).reshape(ROWS, IN_F)
    in_maps = []
    for k in range(NCORES):
        b = (k * ROWS_PER_CORE) // N  # batch this core's rows belong to
        in_maps.append(
            {
                "x": xf[k * ROWS_PER_CORE : (k + 1) * ROWS_PER_CORE],
                "wt": wmodT[b],
                "betar": betar[b],
                "ident": ident,
            }
        )
    return in_maps


def kernel(x, z, weight, weight_alpha, bias_alpha, weight_beta, bias_beta,
           _trace=False):
    from concourse.bass_utils import run_bass_kernel_spmd

    in_maps = host_prep(
        x, z, weight, weight_alpha, bias_alpha, weight_beta, bias_beta
    )
    nc = _get_nc()
    res = run_bass_kernel_spmd(
        nc, in_maps, core_ids=list(range(NCORES)), trace=_trace
    )
    out = np.concatenate([res.results[k]["out"] for k in range(NCORES)], axis=0)
    out = out.reshape(B, N, OUT_F)
    if _trace:
        kernel.last_results = res
    return out


# revision 6
# speedup vs baseline: 1.1075x; 1.0894x over previous
"""Trainium2 Bass kernel for ModLinear forward:

    alpha = z @ weight_alpha.T + bias_alpha          # [B, IN]
    beta  = z @ weight_beta.T  + bias_beta           # [B, OUT]
    out   = (x * alpha[:, None, :]) @ weight.T + beta[:, None, :]

Key restructuring: alpha modulates the *input channels*, so it folds into the
weight per batch:  out[b] = x[b] @ (weight.T * alpha[b][:, None]) + beta[b].
The huge x tensor is then consumed by a plain matmul with a per-batch
pre-modulated weight (tiny, computed on host along with alpha/beta).

Sharding: x is flattened to [B*N, IN] and split into 8 contiguous row blocks
(one per NeuronCore); batch boundary falls exactly between cores 3 and 4, so
each core uses a single (wmodT, beta) pair. No cross-core communication.

Device kernel per core (rows = 32768):
  for each 128-row block:
    DMA x block [128, 512] -> SBUF
    4x PE-transpose 128x128  -> PSUM   (feature dim onto partitions)
    ACT copy PSUM -> SBUF  (xT chunks)
    4x PE matmul (f32r, accumulate over the 4 feature chunks) -> PSUM [128, 512]
    DVE add beta (broadcast pre-replicated on host) -> SBUF
    DMA out block -> DRAM
"""

import numpy as np

B, N = 2, 131072
IN_F, OUT_F, STYLE_F = 512, 512, 256
NCORES = 8
ROWS = B * N
ROWS_PER_CORE = ROWS // NCORES  # 32768
P = 128


def _build_body(tc, out_ap, x_ap, wt_ap, betar_ap, ident_ap, rows_per_core):
    import concourse.bass as bass
    from concourse import mybir

    nc = tc.nc
    f32 = mybir.dt.float32
    f32r = mybir.dt.float32r
    # Superblock: V*128 rows, packed as [128 partitions, V rows x 512 feat].
    # Partition u holds DRAM rows (V*u .. V*u+V-1) -> V*2 KiB contiguous per
    # partition -> big DMA descriptors, V*512 KiB per dma_start.
    V = 8
    SB = V * P
    nsuper = rows_per_core // SB

    x_v = x_ap.rearrange("(s u v) i -> s u (v i)", u=P, v=V)
    out_v = out_ap.rearrange("(s u v) o -> s u (v o)", u=P, v=V)

    with (
        tc.tile_pool(name="const", bufs=1) as cpool,
        tc.tile_pool(name="xin", bufs=2) as xpool,
        tc.tile_pool(name="xt", bufs=4) as xtpool,
        tc.tile_pool(name="oout", bufs=2) as opool,
        tc.tile_pool(name="ptr", bufs=2, space="PSUM") as ptpool,
        tc.tile_pool(name="pmm", bufs=2, space="PSUM") as pmpool,
    ):
        # Constants: modulated transposed weight (4 chunks of [128, 512] side
        # by side), replicated beta, 128x128 identity for PE transpose.
        wt_sb = cpool.tile([P, 4 * OUT_F], f32r)
        for c in range(4):
            nc.sync.dma_start(
                out=wt_sb[:, c * OUT_F : (c + 1) * OUT_F],
                in_=wt_ap[c * P : (c + 1) * P, :],
            )
        beta_sb = cpool.tile([P, OUT_F], f32)
        nc.sync.dma_start(out=beta_sb[:], in_=betar_ap[:, :])
        ident_sb = cpool.tile([P, P], f32r)
        nc.sync.dma_start(out=ident_sb[:], in_=ident_ap[:, :])

        for s in range(nsuper):
            xt = xpool.tile([P, V * IN_F], f32r)
            nc.sync.dma_start(out=xt[:], in_=x_v[s])
            ot = opool.tile([P, V * OUT_F], f32)

            # 4 passes of 2 row-groups each (PSUM: 2+2 banks, double-buffered)
            for h in range(V // 2):
                pt = ptpool.tile([P, 2 * IN_F], f32r)
                for gg in range(2):
                    v = 2 * h + gg
                    for c in range(4):
                        nc.tensor.transpose(
                            pt[:, gg * IN_F + c * P : gg * IN_F + (c + 1) * P],
                            xt[:, v * IN_F + c * P : v * IN_F + (c + 1) * P],
                            ident_sb[:],
                        )
                xts = xtpool.tile([P, 2 * IN_F], f32r)
                nc.scalar.copy(out=xts[:, :IN_F], in_=pt[:, :IN_F])
                nc.scalar.copy(out=xts[:, IN_F:], in_=pt[:, IN_F:])

                po = pmpool.tile([P, 2 * OUT_F], f32)
                for gg in range(2):
                    for c in range(4):
                        nc.tensor.matmul(
                            po[:, gg * OUT_F : (gg + 1) * OUT_F],
                            xts[:, gg * IN_F + c * P : gg * IN_F + (c + 1) * P],
                            wt_sb[:, c * OUT_F : (c + 1) * OUT_F],
                            start=(c == 0),
                            stop=(c == 3),
                        )

                for gg in range(2):
                    v = 2 * h + gg
                    nc.vector.tensor_add(
                        out=ot[:, v * OUT_F : (v + 1) * OUT_F],
                        in0=po[:, gg * OUT_F : (gg + 1) * OUT_F],
                        in1=beta_sb[:],
                    )

            nc.scalar.dma_start(out=out_v[s], in_=ot[:])


def build_nc(rows_per_core=ROWS_PER_CORE):
    """Build + compile the per-core Bass program. Returns (nc, names)."""
    import concourse.tile as tile
    from concourse import bacc, mybir

    f32 = mybir.dt.float32
    f32r = mybir.dt.float32r
    nc = bacc.Bacc(
        "TRN2", target_bir_lowering=False, debug=False, num_devices=NCORES
    )
    x_t = nc.dram_tensor("x", [rows_per_core, IN_F], f32r, kind="ExternalInput")
    wt_t = nc.dram_tensor("wt", [IN_F, OUT_F], f32r, kind="ExternalInput")
    betar_t = nc.dram_tensor("betar", [P, OUT_F], f32, kind="ExternalInput")
    ident_t = nc.dram_tensor("ident", [P, P], f32r, kind="ExternalInput")
    out_t = nc.dram_tensor("out", [rows_per_core, OUT_F], f32, kind="ExternalOutput")

    with tile.TileContext(nc) as tc:
        _build_body(
            tc, out_t.ap(), x_t.ap(), wt_t.ap(), betar_t.ap(), ident_t.ap(),
            rows_per_core,
        )
    nc.compile()
    return nc


_NC_CACHE = {}


def _get_nc(rows_per_core=ROWS_PER_CORE):
    if rows_per_core not in _NC_CACHE:
        _NC_CACHE[rows_per_core] = build_nc(rows_per_core)
    return _NC_CACHE[rows_per_core]


def host_prep(x, z, weight, weight_alpha, bias_alpha, weight_beta, bias_beta):
    """Compute per-batch modulated weights + biases, and per-core in_maps."""
    z64 = z.astype(np.float64)
    alpha = (z64 @ weight_alpha.astype(np.float64).T) + bias_alpha.astype(np.float64)
    beta = (z64 @ weight_beta.astype(np.float64).T) + bias_beta.astype(np.float64)
    alpha = alpha.astype(np.float32)  # [B, IN_F]
    beta = beta.astype(np.float32)  # [B, OUT_F]

    # wmodT[b][i, o] = weight[o, i] * alpha[b, i]
    wmodT = [
        np.ascontiguousarray(weight.T * alpha[b][:, None]).astype(np.float32)
        for b in range(B)
    ]
    betar = [
        np.ascontiguousarray(np.broadcast_to(beta[b], (P, OUT_F))).astype(np.float32)
        for b in range(B)
    ]
    ident = np.eye(P, dtype=np.float32)

    xf = np.ascontiguousarray(x).reshape(ROWS, IN_F)
    in_maps = []
    for k in range(NCORES):
        b = (k * ROWS_PER_CORE) // N  # batch this core's rows belong to
        in_maps.append(
            {
                "x": xf[k * ROWS_PER_CORE : (k + 1) * ROWS_PER_CORE],
                "wt": wmodT[b],
                "betar": betar[b],
                "ident": ident,
            }
        )
    return in_maps


def kernel(x, z, weight, weight_alpha, bias_alpha, weight_beta, bias_beta,
           _trace=False):
    from concourse.bass_utils import run_bass_kernel_spmd

    in_maps = host_prep(
        x, z, weight, weight_alpha, bias_alpha, weight_beta, bias_beta
    )
    nc = _get_nc()
    res = run_bass_kernel_spmd(
        nc, in_maps, core_ids=list(range(NCORES)), trace=_trace
    )
    out = np.concatenate([res.results[k]["out"] for k in range(NCORES)], axis=0)
    out = out.reshape(B, N, OUT_F)
    if _trace:
        kernel.last_results = res
    return out
